# revision 3
# baseline (speedup 1.0000x reference)
"""DIN attention kernel for Trainium2 (8 NeuronCores, data-parallel over batch).

Math (per batch row b, position s):
  din  = [t, seq, t-seq, t*seq]  -> relu MLP 256->80->40->1 -> masked softmax over s.

Key structure:
- Weight fold: din @ W1 = seq @ Ws' + (t*seq) @ Wm + U with
  U = t @ Wt' + b1 per row. The per-row U term is folded into the
  shipped data on the host: solve Wsm^T z_row = U_row (least-norm,
  against the bf16-rounded device weights) and add z_row to every
  gathered column, so mm1 produces ps1 + U directly.
- The elementwise product t*seq is precomputed on the host during the
  gather, so din = [seq+z1; t*seq+z2] arrives via two plain 2D DMAs.
- Mask sparsity + variable-width tiers: rows are sorted per-core by
  unmasked count and grouped into 4 superblocks of 128 rows; each
  superblock is gathered to its own column cap (the max count in the
  group, rounded up to 8, <= 128). Padding columns get an additive
  -1e9 mask so their exp is exactly 0. The host normalizes (divide by
  the shipped per-row exp-sum) and scatters back to [B, 200].
- On-device per 16-row chunk: mm1 (K=128, M=80) into two [80, 2, 4*cap]
  PSUM tiles, mm2 2-way column-tiled into [128, 4*cap] pair tiles, and
  M=32 shifted-column w3 matmuls (4-way tiled) accumulating scores into
  [128, cap] PSUM tiles per superblock.
- PSUM evacuation is split into two parallel lanes: ACT does relu(ps1a)
  and relu(ps2_pair0 + b2), DVE does relu(ps1b) and relu(ps2_pair1 +
  b2). The additive -1e9 mask is folded into the score accumulation as
  one identity-weighted matmul on the PE, and softmax is reduced to a
  single exp+accumulate ACT op per superblock (scores are bounded,
  |s| < 6, so no max subtraction); the divide happens on the host.
- Batch rows are processed in a permuted order (dev row 4r+q <-> score
  row 32q+r) composed with the count-sort; host arrays undo both.
"""

import sys

sys.path.insert(0, "/opt/trn_rl_repo")

import numpy as np
import ml_dtypes

B, S, D = 4096, 200, 64
H1, H2 = 80, 40
NCORES = 8
BPC = B // NCORES          # 512 batch rows per core
CAPMAX = 128
CHUNK_B = 16               # batch rows per chunk (4 quads)
NCHUNK = BPC // CHUNK_B    # 32
NSUPER = BPC // 128        # 4 superblocks (128 rows each)

_cache = {}


def _build_nc(caps):
    import concourse.bass as bass
    import concourse.mybir as mybir
    import concourse.tile as tile
    from concourse import bacc

    f32 = mybir.dt.float32
    bf16 = mybir.dt.bfloat16
    AF = mybir.ActivationFunctionType
    ALU = mybir.AluOpType

    def cap(c):
        return caps[c // 8]

    TOT = sum(CHUNK_B * cap(c) for c in range(NCHUNK))
    offs = np.cumsum([0] + [CHUNK_B * cap(c) for c in range(NCHUNK)])

    nc = bacc.Bacc(None, target_bir_lowering=False)

    seq_d = nc.declare_dram_parameter("seqg", [D, TOT], bf16, isOutput=False)
    prod_d = nc.declare_dram_parameter("prodg", [D, TOT], bf16, isOutput=False)
    wsm_d = nc.declare_dram_parameter("wsm", [2 * D, H1], bf16, isOutput=False)
    w2_d = nc.declare_dram_parameter("w2", [H1, 64], bf16, isOutput=False)
    w3s_d = nc.declare_dram_parameter("w3s2", [128, 64], bf16, isOutput=False)
    b2_d = nc.declare_dram_parameter("b2e", [128, 1], f32, isOutput=False)
    am_d = nc.declare_dram_parameter("amaskb", [BPC, CAPMAX], bf16, isOutput=False)
    id_d = nc.declare_dram_parameter("ident", [128, 128], bf16, isOutput=False)
    out_d = nc.declare_dram_parameter("out", [BPC, CAPMAX], f32, isOutput=True)
    sum_d = nc.declare_dram_parameter("sums", [BPC, 1], f32, isOutput=True)

    def din_src(d_t, c):
        return d_t[:, offs[c]:offs[c + 1]].rearrange(
            "d (k t) -> d k t", k=CHUNK_B)

    with tile.TileContext(nc) as tc:
        with (
            tc.tile_pool(name="singles", bufs=1) as singles,
            tc.tile_pool(name="dinpool", bufs=3) as dinpool,
            tc.tile_pool(name="h1pool", bufs=4) as h1pool,
            tc.tile_pool(name="h2pool", bufs=4) as h2pool,
            tc.tile_pool(name="smpool", bufs=2) as smpool,
            tc.tile_pool(name="ps1pool", bufs=1, space="PSUM") as ps1pool,
            tc.tile_pool(name="ps2pool", bufs=2, space="PSUM") as ps2pool,
            tc.tile_pool(name="scpool", bufs=2, space="PSUM") as scpool,
        ):
            # mm1 weights first (tiny), then chunk 0 split per quad across
            # four DMA queues so the first mm1 only waits ~one quad's bytes
            wsm = singles.tile([2 * D, H1], bf16)
            nc.sync.dma_start(out=wsm, in_=wsm_d[:])
            c0 = cap(0)
            din0 = dinpool.tile([128, CHUNK_B, c0], bf16, tag="din")
            s0 = din_src(seq_d, 0)
            p0 = din_src(prod_d, 0)
            seq_q = [nc.sync, nc.scalar, nc.vector, nc.sync]
            for qq in range(4):
                seq_q[qq].dma_start(
                    out=din0[0:D, 4 * qq:4 * qq + 4, :],
                    in_=s0[:, 4 * qq:4 * qq + 4, :])
                nc.gpsimd.dma_start(
                    out=din0[D:128, 4 * qq:4 * qq + 4, :],
                    in_=p0[:, 4 * qq:4 * qq + 4, :])
            din1 = dinpool.tile([128, CHUNK_B, cap(1)], bf16, tag="din")
            nc.scalar.dma_start(out=din1[0:D, :, :], in_=din_src(seq_d, 1))
            nc.gpsimd.dma_start(out=din1[D:128, :, :], in_=din_src(prod_d, 1))
            w2 = singles.tile([H1, 64], bf16)
            nc.sync.dma_start(out=w2, in_=w2_d[:])
            w3s2 = singles.tile([128, 64], bf16)
            nc.sync.dma_start(out=w3s2, in_=w3s_d[:])
            b2e = singles.tile([128, 1], f32)
            nc.sync.dma_start(out=b2e, in_=b2_d[:])
            ident = singles.tile([128, 128], bf16)
            nc.sync.dma_start(out=ident, in_=id_d[:])

            # HAM warm-up: the PE clock-gate only opens (1.2 -> 2.4 GHz) after
            # ~3.4us of sustained matmul activity, and the first real matmul
            # waits on the input DMA pipe anyway. Fill that window with dummy
            # matmuls into the sb0 score tile (whose first real write uses
            # start=True, so the garbage is never observed).
            zt = singles.tile([128, 512], bf16)
            nc.vector.memset(zt, 0.0)
            # trigger the ACT spline-table load (~1.3us) before chunk 0's relu
            nc.scalar.activation(zt[0:1, 0:1], zt[0:1, 0:1], AF.Relu)
            sc_tiles = {}
            sc_tiles[0] = scpool.tile([128, cap(0)], f32, name="scA")
            for w in range(32):
                nc.tensor.matmul(
                    sc_tiles[0], lhsT=zt[:, 0:128], rhs=zt[:, 0:cap(0)],
                    start=True, stop=(w >= 31), skip_group_check=True)

            # software-pipelined: per chunk c emit mm1/relu/mm2/h2 for c and
            # the score matmuls for c-1, so the in-order tensor queue is
            # [mm1 x4, mm3 (prev), mm2] and never stalls on the relu.
            h2q = {}

            def emit_exp(sb):
                scA = sc_tiles.pop(sb)
                expm = smpool.tile([128, caps[sb]], f32)
                sume = smpool.tile([128, 1], f32)
                nc.scalar.activation(expm, scA, AF.Exp, accum_out=sume)
                nc.sync.dma_start(
                    out=out_d[sb * 128:(sb + 1) * 128, 0:caps[sb]], in_=expm)
                nc.sync.dma_start(out=sum_d[sb * 128:(sb + 1) * 128, :],
                                  in_=sume)

            amts = {}
            amt_done = set()
            h1q = {}

            def emit_mm2(c, pair):
                # one mm2 round: quads (2*pair, 2*pair+1), 2-way col-tiled
                n4 = 4 * cap(c)
                ps2 = ps2pool.tile([128, n4], f32, name="ps2")
                for sub in range(2):
                    rb = 0 if sub == 0 else 64
                    nc.tensor.matmul(ps2[rb:rb + H2, :],
                                     lhsT=w2[:, 0:H2],
                                     rhs=h1q[(c, pair)][:, sub, :],
                                     start=True, stop=True,
                                     tile_position=(0, rb))
                h2t = h2pool.tile([128, n4], bf16, tag="h2")
                if pair == 0:
                    nc.scalar.activation(h2t[0:104, :], ps2[0:104, :],
                                         AF.Relu, bias=b2e[0:104, 0:1])
                else:
                    nc.vector.tensor_scalar(
                        h2t[0:104, :], ps2[0:104, :],
                        b2e[0:104, 0:1], 0.0,
                        op0=ALU.add, op1=ALU.max)
                h2q[(c, pair)] = h2t

            def emit_pack(c, pair):
                sb, ch = c // 8, c % 8
                cp = caps[sb]
                if sb not in sc_tiles:
                    sc_tiles[sb] = scpool.tile([128, cp], f32, name="scA")
                scA = sc_tiles[sb]
                if sb not in amt_done:
                    # fold the additive -1e9 pad mask into the score
                    # accumulation: scA = I^T @ amask, then packs accumulate
                    amt_done.add(sb)
                    nc.tensor.matmul(
                        scA, lhsT=ident, rhs=amts.pop(sb),
                        start=True, stop=False, skip_group_check=True)
                h2t = h2q.pop((c, pair))
                ra = ch * 4 + 2 * pair
                last = (ch == 7 and pair == 1)
                for q in range(4):
                    nc.tensor.matmul(
                        scA[32 * q:32 * q + 32, :],
                        lhsT=w3s2[0:104, 32 - ra:64 - ra],
                        rhs=h2t[0:104, q * cp:(q + 1) * cp],
                        start=False, stop=last,
                        tile_position=(0, 32 * q),
                        skip_group_check=True)

            # half-chunk rotated pipeline: the tensor queue per iteration is
            # [mm1-q01(c), mm2-q23(c-1), mm1-q23(c), pack-p0(c-1),
            #  mm2-q01(c), pack-p1(c-1)], so mm2 never waits on the relu of
            # its own half and the PE stays streaming.
            for c in range(NCHUNK):
                sb, ch = c // 8, c % 8
                cp = cap(c)
                if ch == 0:
                    amt = smpool.tile([128, cp], bf16)
                    nc.sync.dma_start(
                        out=amt, in_=am_d[sb * 128:(sb + 1) * 128, 0:cp])
                    amts[sb] = amt
                if c == 0:
                    din = din0
                elif c == 1:
                    din = din1
                else:
                    din = dinpool.tile([128, CHUNK_B, cp], bf16, tag="din")
                    nc.sync.dma_start(out=din[0:D, :, :],
                                      in_=din_src(seq_d, c))
                    nc.gpsimd.dma_start(out=din[D:128, :, :],
                                        in_=din_src(prod_d, c))

                ps1a = ps1pool.tile([H1, 2, 4 * cp], f32)
                ps1b = ps1pool.tile([H1, 2, 4 * cp], f32)
                for qq in range(2):
                    nc.tensor.matmul(
                        ps1a[:, qq, :],
                        lhsT=wsm, rhs=din[:, 4 * qq:4 * qq + 4, :],
                        start=True, stop=True, skip_group_check=True)
                h1a = h1pool.tile([H1, 2, 4 * cp], bf16, tag="h1a")
                nc.scalar.activation(h1a, ps1a, AF.Relu)
                h1q[(c, 0)] = h1a

                if c > 0:
                    emit_mm2(c - 1, 1)

                for qq in range(2, 4):
                    nc.tensor.matmul(
                        ps1b[:, qq % 2, :],
                        lhsT=wsm, rhs=din[:, 4 * qq:4 * qq + 4, :],
                        start=True, stop=True, skip_group_check=True)
                h1b = h1pool.tile([H1, 2, 4 * cp], bf16, tag="h1b")
                nc.vector.tensor_scalar_max(h1b, ps1b, 0.0)
                h1q[(c, 1)] = h1b

                if c > 0:
                    emit_pack(c - 1, 0)
                emit_mm2(c, 0)
                if c > 0:
                    emit_pack(c - 1, 1)
                    if ch == 0:
                        emit_exp(sb - 1)

            c = NCHUNK - 1
            emit_mm2(c, 1)
            emit_pack(c, 0)
            emit_pack(c, 1)
            emit_exp(NSUPER - 1)

    nc.finalize()
    return nc


def _host_prep(inputs):
    bf16 = ml_dtypes.bfloat16
    seq = np.asarray(inputs["sequence_emb"], dtype=np.float32)
    tgt = np.asarray(inputs["target_emb"], dtype=np.float32)
    mask = np.asarray(inputs["mask"])
    W1 = np.asarray(inputs["W1"], dtype=np.float32)
    b1 = np.asarray(inputs["b1"], dtype=np.float32)
    W2 = np.asarray(inputs["W2"], dtype=np.float32)
    b2 = np.asarray(inputs["b2"], dtype=np.float32)
    W3 = np.asarray(inputs["W3"], dtype=np.float32)

    Wt = W1[0:64] + W1[128:192]
    Ws = W1[64:128] - W1[128:192]
    Wm = W1[192:256]
    wsm = np.concatenate([Ws, Wm], axis=0).astype(bf16)
    # Fold U = t @ Wt' + b1 into the shipped data: solve wsm^T z = U
    # (least-norm, against the bf16-rounded device weights).
    wsm_f = wsm.astype(np.float32)
    Zmap = (wsm_f @ np.linalg.inv(wsm_f.T @ wsm_f)).astype(np.float32)  # [128, H1]
    U = tgt @ Wt + b1                                   # [B, H1] f32
    Z = U @ Zmap.T                                      # [B, 128]
    w2 = np.zeros((H1, 64), dtype=bf16)
    w2[:, 0:H2] = W2.astype(bf16)
    w3s2 = np.zeros((128, 64), dtype=bf16)
    w3s2[0:H2, 32] = W3[:, 0].astype(bf16)
    w3s2[64:64 + H2, 33] = W3[:, 0].astype(bf16)
    b2e = np.zeros((128, 1), dtype=np.float32)
    b2e[0:H2, 0] = b2
    b2e[64:64 + H2, 0] = b2

    maskb = mask.astype(bool)
    cnt = maskb.sum(1).astype(np.int64)
    assert cnt.max() <= CAPMAX, f"unmasked count {cnt.max()} exceeds {CAPMAX}"

    # sort rows by count within each core; superblock sb takes the sb-th
    # 128-row group, so its column cap is the group's max count (shared
    # across cores, rounded up to 8)
    cnt_c = cnt.reshape(NCORES, BPC)
    order_c = np.argsort(cnt_c, axis=1, kind="stable")   # [NCORES, BPC]
    caps = []
    for sb in range(NSUPER):
        grp = np.take_along_axis(
            cnt_c, order_c[:, sb * 128:(sb + 1) * 128], axis=1)
        caps.append(min(CAPMAX, int(-(-int(grp.max()) // 8) * 8)))
    caps = tuple(caps)

    # device row permutation: count-sort composed with the per-128-block
    # pack permutation dev 4r+q <-> 32q+r
    r_ = np.arange(128) // 4
    q_ = np.arange(128) % 4
    perm128 = 32 * q_ + r_
    assign = np.concatenate([
        core * BPC + order_c[core][sb * 128 + perm128]
        for core in range(NCORES) for sb in range(NSUPER)])   # [B] dev->orig

    # gather indices: unmasked positions first, padded with a masked slot
    order = np.argsort(~maskb, axis=1, kind="stable")   # unmasked first
    idx = order[:, :CAPMAX]
    pad_slot = order[:, -1]                             # guaranteed masked
    colpos = np.arange(CAPMAX)[None, :]
    idx = np.where(colpos < cnt[:, None], idx, pad_slot[:, None])

    # gathered feature-major seq+z1 and t*seq+z2 in device row order with
    # per-superblock widths, flattened to [64, TOT] per core
    g = np.take_along_axis(seq, idx[:, :, None], axis=1)     # [B, CAPMAX, D]
    sg = (g + Z[:, None, 0:64]).transpose(0, 2, 1).astype(bf16)       # [B, D, CAPMAX]
    pg = (g * tgt[:, None, :] + Z[:, None, 64:128]).transpose(0, 2, 1).astype(bf16)
    amask = np.where(colpos < cnt[:, None], 0.0, -1e9).astype(bf16)
    ident = np.eye(128, dtype=bf16)

    assign_c = assign.reshape(NCORES, BPC)
    TOT = CHUNK_B * sum(caps[c // 8] * 8 for c in range(NSUPER))  # per-core cols

    def flat_core(x, core):                      # x: [B, D, CAPMAX] -> [D, TOT]
        parts = []
        rows = assign_c[core]
        for c in range(NCHUNK):
            cp = caps[c // 8]
            blk = x[rows[c * CHUNK_B:(c + 1) * CHUNK_B], :, 0:cp]  # [16, D, cp]
            parts.append(blk.transpose(1, 0, 2).reshape(D, CHUNK_B * cp))
        return np.ascontiguousarray(np.concatenate(parts, axis=1))

    in_maps = []
    for core in range(NCORES):
        in_maps.append({
            "seqg": flat_core(sg, core),
            "prodg": flat_core(pg, core),
            "wsm": wsm,
            "w2": w2,
            "w3s2": w3s2,
            "b2e": b2e,
            "amaskb": amask[assign_c[core]],
            "ident": ident,
        })
    return in_maps, idx, assign, caps


def kernel(**inputs) -> np.ndarray:
    from concourse.bass_utils import run_bass_kernel_spmd

    in_maps, idx, assign, caps = _host_prep(inputs)
    if caps not in _cache:
        _cache[caps] = _build_nc(caps)
    nc = _cache[caps]
    res = run_bass_kernel_spmd(nc, in_maps, list(range(NCORES)))
    expm = np.concatenate(
        [res.results[i]["out"] for i in range(NCORES)], axis=0)   # [B, CAPMAX] dev order
    sums = np.concatenate(
        [res.results[i]["sums"] for i in range(NCORES)], axis=0)  # [B, 1]
    probs = expm / sums
    out = np.zeros((B, S), dtype=np.float32)
    out[assign[:, None], idx[assign]] = probs
    return out


if __name__ == "__main__":
    rng = np.random.default_rng(0)
    fake = {
        "sequence_emb": rng.standard_normal((B, S, D), dtype=np.float32),
        "target_emb": rng.standard_normal((B, D), dtype=np.float32),
        "mask": rng.integers(0, 2, (B, S)).astype(np.int32),
        "W1": rng.standard_normal((4 * D, H1), dtype=np.float32) * 0.08,
        "b1": np.zeros(H1, np.float32),
        "W2": rng.standard_normal((H1, H2), dtype=np.float32) * 0.13,
        "b2": np.zeros(H2, np.float32),
        "W3": rng.standard_normal((H2, 1), dtype=np.float32) * 0.22,
        "b3": np.zeros(1, np.float32),
    }
    print(kernel(**fake).shape)


# revision 11
# speedup vs baseline: 1.1441x; 1.1441x over previous
"""DIN attention kernel for Trainium2 (8 NeuronCores, data-parallel over batch).

Math (per batch row b, position s):
  din  = [t, seq, t-seq, t*seq]  -> relu MLP 256->80->40->1 -> masked softmax over s.

Key structure:
- Weight fold: din @ W1 = seq @ Ws' + (t*seq) @ Wm + U with
  U = t @ Wt' + b1 per row. The per-row U term is folded into the
  shipped data on the host: solve Wsm^T z_row = U_row (least-norm,
  against the bf16-rounded device weights) and add z_row to every
  gathered column, so mm1 produces ps1 + U directly.
- The elementwise product t*seq is precomputed on the host during the
  gather, so din = [seq+z1; t*seq+z2] arrives via two plain 2D DMAs.
- Mask sparsity + variable-width tiers: rows are sorted per-core by
  unmasked count and grouped into 4 superblocks of 128 rows; each
  superblock is gathered to its own column cap (the max count in the
  group, rounded up to 8, <= 128). Padding columns get an additive
  -1e9 mask so their exp is exactly 0. The host normalizes (divide by
  the shipped per-row exp-sum) and scatters back to [B, 200].
- On-device per 16-row chunk: mm1 (K=128, M=80) into two [80, 2, 4*cap]
  PSUM tiles, mm2 2-way column-tiled into [128, 4*cap] pair tiles, and
  M=32 shifted-column w3 matmuls (4-way tiled) accumulating scores into
  [128, cap] PSUM tiles per superblock.
- PSUM evacuation is split into two parallel lanes: ACT does relu(ps1a)
  and relu(ps2_pair0 + b2), DVE does relu(ps1b) and relu(ps2_pair1 +
  b2). The additive -1e9 mask is folded into the score accumulation as
  one identity-weighted matmul on the PE, and softmax is reduced to a
  single exp+accumulate ACT op per superblock (scores are bounded,
  |s| < 6, so no max subtraction); the divide happens on the host.
- Batch rows are processed in a permuted order (dev row 4r+q <-> score
  row 32q+r) composed with the count-sort; host arrays undo both.
"""

import sys

sys.path.insert(0, "/opt/trn_rl_repo")

import numpy as np
import ml_dtypes

B, S, D = 4096, 200, 64
H1, H2 = 80, 40
NCORES = 8
BPC = B // NCORES          # 512 batch rows per core
CAPMAX = 128
CHUNK_B = 16               # batch rows per chunk (4 quads)
NCHUNK = BPC // CHUNK_B    # 32
NSUPER = BPC // 128        # 4 superblocks (128 rows each)

_cache = {}


def _build_nc(caps):
    import concourse.bass as bass
    import concourse.mybir as mybir
    import concourse.tile as tile
    from concourse import bacc

    f32 = mybir.dt.float32
    bf16 = mybir.dt.bfloat16
    AF = mybir.ActivationFunctionType
    ALU = mybir.AluOpType

    def cap(c):
        return caps[c // 8]

    TOT = sum(CHUNK_B * cap(c) for c in range(NCHUNK))
    offs = np.cumsum([0] + [CHUNK_B * cap(c) for c in range(NCHUNK)])

    nc = bacc.Bacc(None, target_bir_lowering=False)

    seq_d = nc.declare_dram_parameter("seqg", [D, TOT], bf16, isOutput=False)
    prod_d = nc.declare_dram_parameter("prodg", [D, TOT], bf16, isOutput=False)
    wsm_d = nc.declare_dram_parameter("wsm", [2 * D, H1], bf16, isOutput=False)
    w2_d = nc.declare_dram_parameter("w2", [H1, 64], bf16, isOutput=False)
    w3s_d = nc.declare_dram_parameter("w3s2", [128, 64], bf16, isOutput=False)
    b2_d = nc.declare_dram_parameter("b2e", [128, 1], f32, isOutput=False)
    am_d = nc.declare_dram_parameter("amaskb", [BPC, CAPMAX], bf16, isOutput=False)
    id_d = nc.declare_dram_parameter("ident", [128, 128], bf16, isOutput=False)
    out_d = nc.declare_dram_parameter("out", [BPC, CAPMAX], f32, isOutput=True)
    sum_d = nc.declare_dram_parameter("sums", [BPC, 1], f32, isOutput=True)

    def din_src(d_t, c):
        return d_t[:, offs[c]:offs[c + 1]].rearrange(
            "d (k t) -> d k t", k=CHUNK_B)

    with tile.TileContext(nc) as tc:
        with (
            tc.tile_pool(name="singles", bufs=1) as singles,
            tc.tile_pool(name="dinpool", bufs=3) as dinpool,
            tc.tile_pool(name="h1pool", bufs=4) as h1pool,
            tc.tile_pool(name="h2pool", bufs=4) as h2pool,
            tc.tile_pool(name="smpool", bufs=2) as smpool,
            tc.tile_pool(name="ps1pool", bufs=1, space="PSUM") as ps1pool,
            tc.tile_pool(name="ps2pool", bufs=2, space="PSUM") as ps2pool,
            tc.tile_pool(name="scpool", bufs=2, space="PSUM") as scpool,
        ):
            # mm1 weights first (tiny), then chunk 0 split per quad across
            # four DMA queues so the first mm1 only waits ~one quad's bytes
            wsm = singles.tile([2 * D, H1], bf16)
            nc.sync.dma_start(out=wsm, in_=wsm_d[:])
            c0 = cap(0)
            din0 = dinpool.tile([128, CHUNK_B, c0], bf16, tag="din")
            s0 = din_src(seq_d, 0)
            p0 = din_src(prod_d, 0)
            seq_q = [nc.sync, nc.scalar, nc.sync, nc.scalar]
            for qq in range(4):
                seq_q[qq].dma_start(
                    out=din0[0:D, 4 * qq:4 * qq + 4, :],
                    in_=s0[:, 4 * qq:4 * qq + 4, :])
                nc.gpsimd.dma_start(
                    out=din0[D:128, 4 * qq:4 * qq + 4, :],
                    in_=p0[:, 4 * qq:4 * qq + 4, :])
            din1 = dinpool.tile([128, CHUNK_B, cap(1)], bf16, tag="din")
            nc.scalar.dma_start(out=din1[0:D, :, :], in_=din_src(seq_d, 1))
            nc.gpsimd.dma_start(out=din1[D:128, :, :], in_=din_src(prod_d, 1))
            w2 = singles.tile([H1, 64], bf16)
            nc.sync.dma_start(out=w2, in_=w2_d[:])
            w3s2 = singles.tile([128, 64], bf16)
            nc.sync.dma_start(out=w3s2, in_=w3s_d[:])
            b2e = singles.tile([128, 1], f32)
            nc.sync.dma_start(out=b2e, in_=b2_d[:])
            ident = singles.tile([128, 128], bf16)
            nc.sync.dma_start(out=ident, in_=id_d[:])

            # HAM warm-up: the PE clock-gate only opens (1.2 -> 2.4 GHz) after
            # ~3.4us of sustained matmul activity, and the first real matmul
            # waits on the input DMA pipe anyway. Fill that window with dummy
            # matmuls into the sb0 score tile (whose first real write uses
            # start=True, so the garbage is never observed).
            zt = singles.tile([128, 512], bf16)
            nc.vector.memset(zt, 0.0)
            # trigger the ACT spline-table load (~1.3us) before chunk 0's relu
            nc.scalar.activation(zt[0:1, 0:1], zt[0:1, 0:1], AF.Relu)
            sc_tiles = {}
            sc_tiles[0] = scpool.tile([128, cap(0)], f32, name="scA")
            for w in range(32):
                nc.tensor.matmul(
                    sc_tiles[0], lhsT=zt[:, 0:128], rhs=zt[:, 0:cap(0)],
                    start=True, stop=(w >= 31), skip_group_check=True)

            # software-pipelined: per chunk c emit mm1/relu/mm2/h2 for c and
            # the score matmuls for c-1, so the in-order tensor queue is
            # [mm1 x4, mm3 (prev), mm2] and never stalls on the relu.
            h2q = {}

            def emit_exp(sb):
                scA = sc_tiles.pop(sb)
                expm = smpool.tile([128, caps[sb]], f32)
                sume = smpool.tile([128, 1], f32)
                nc.scalar.activation(expm, scA, AF.Exp, accum_out=sume)
                nc.sync.dma_start(
                    out=out_d[sb * 128:(sb + 1) * 128, 0:caps[sb]], in_=expm)
                nc.sync.dma_start(out=sum_d[sb * 128:(sb + 1) * 128, :],
                                  in_=sume)

            amts = {}
            amt_done = set()
            h1q = {}

            def emit_mm2(c, pair):
                # one mm2 round: quads (2*pair, 2*pair+1), 2-way col-tiled
                n4 = 4 * cap(c)
                ps2 = ps2pool.tile([128, n4], f32, name="ps2")
                # HAM keep-warm filler: the clock-gate demotes the PE to 4/8
                # (1.2 GHz) whenever its idle fraction over the ~3.4us activity
                # window grows, and never re-promotes mid-kernel. Burn the
                # h1-relu wait with a dependency-free zero matmul over
                # ps2[0:104]; the real mm2s then overwrite rows 0-39/64-103
                # (start=True) and rows 40-63 read back as clean zeros.
                nc.tensor.matmul(ps2[0:104, :], lhsT=zt[:, 0:104],
                                 rhs=zt[:, 0:n4], start=True, stop=True,
                                 skip_group_check=True)
                for sub in range(2):
                    rb = 0 if sub == 0 else 64
                    nc.tensor.matmul(ps2[rb:rb + H2, :],
                                     lhsT=w2[:, 0:H2],
                                     rhs=h1q[(c, pair)][:, sub, :],
                                     start=True, stop=True,
                                     tile_position=(0, rb))
                h2t = h2pool.tile([128, n4], bf16, tag="h2")
                if pair == 0:
                    nc.scalar.activation(h2t[0:104, :], ps2[0:104, :],
                                         AF.Relu, bias=b2e[0:104, 0:1])
                else:
                    nc.vector.tensor_scalar(
                        h2t[0:104, :], ps2[0:104, :],
                        b2e[0:104, 0:1], 0.0,
                        op0=ALU.add, op1=ALU.max)
                h2q[(c, pair)] = h2t

            def emit_pack(c, pair):
                sb, ch = c // 8, c % 8
                cp = caps[sb]
                if sb not in sc_tiles:
                    sc_tiles[sb] = scpool.tile([128, cp], f32, name="scA")
                scA = sc_tiles[sb]
                if sb not in amt_done:
                    # fold the additive -1e9 pad mask into the score
                    # accumulation: scA = I^T @ amask, then packs accumulate
                    amt_done.add(sb)
                    nc.tensor.matmul(
                        scA, lhsT=ident, rhs=amts.pop(sb),
                        start=True, stop=False, skip_group_check=True)
                h2t = h2q.pop((c, pair))
                ra = ch * 4 + 2 * pair
                last = (ch == 7 and pair == 1)
                for q in range(4):
                    nc.tensor.matmul(
                        scA[32 * q:32 * q + 32, :],
                        lhsT=w3s2[0:104, 32 - ra:64 - ra],
                        rhs=h2t[0:104, q * cp:(q + 1) * cp],
                        start=False, stop=last,
                        tile_position=(0, 32 * q),
                        skip_group_check=True)

            # half-chunk rotated pipeline: the tensor queue per iteration is
            # [mm1-q01(c), mm2-q23(c-1), mm1-q23(c), pack-p0(c-1),
            #  mm2-q01(c), pack-p1(c-1)], so mm2 never waits on the relu of
            # its own half and the PE stays streaming.
            for c in range(NCHUNK):
                sb, ch = c // 8, c % 8
                cp = cap(c)
                if ch == 0:
                    amt = smpool.tile([128, cp], bf16)
                    nc.sync.dma_start(
                        out=amt, in_=am_d[sb * 128:(sb + 1) * 128, 0:cp])
                    amts[sb] = amt
                if c == 0:
                    din = din0
                elif c == 1:
                    din = din1
                else:
                    din = dinpool.tile([128, CHUNK_B, cp], bf16, tag="din")
                    nc.sync.dma_start(out=din[0:D, :, :],
                                      in_=din_src(seq_d, c))
                    nc.gpsimd.dma_start(out=din[D:128, :, :],
                                        in_=din_src(prod_d, c))

                # ps1 tiles stay [H1, 2, 512] so each quad's matmul output is
                # bank-aligned: a PSUM matmul output must not cross a 2KB bank
                # boundary, and 4*cp*4B is only bank-aligned at cp=128
                ps1a = ps1pool.tile([H1, 2, 512], f32)
                ps1b = ps1pool.tile([H1, 2, 512], f32)
                for qq in range(2):
                    nc.tensor.matmul(
                        ps1a[:, qq, 0:4 * cp],
                        lhsT=wsm, rhs=din[:, 4 * qq:4 * qq + 4, :],
                        start=True, stop=True, skip_group_check=True)
                h1a = h1pool.tile([H1, 2, 4 * cp], bf16, tag="h1a")
                nc.scalar.activation(h1a, ps1a[:, :, 0:4 * cp], AF.Relu)
                h1q[(c, 0)] = h1a

                if c > 0:
                    emit_mm2(c - 1, 1)

                for qq in range(2, 4):
                    nc.tensor.matmul(
                        ps1b[:, qq % 2, 0:4 * cp],
                        lhsT=wsm, rhs=din[:, 4 * qq:4 * qq + 4, :],
                        start=True, stop=True, skip_group_check=True)
                h1b = h1pool.tile([H1, 2, 4 * cp], bf16, tag="h1b")
                nc.vector.tensor_scalar_max(h1b, ps1b[:, :, 0:4 * cp], 0.0)
                h1q[(c, 1)] = h1b

                if c > 0:
                    emit_pack(c - 1, 0)
                emit_mm2(c, 0)
                if c > 0:
                    emit_pack(c - 1, 1)
                    if ch == 0:
                        emit_exp(sb - 1)

            c = NCHUNK - 1
            emit_mm2(c, 1)
            emit_pack(c, 0)
            emit_pack(c, 1)
            emit_exp(NSUPER - 1)

    nc.finalize()
    return nc


def _host_prep(inputs):
    bf16 = ml_dtypes.bfloat16
    seq = np.asarray(inputs["sequence_emb"], dtype=np.float32)
    tgt = np.asarray(inputs["target_emb"], dtype=np.float32)
    mask = np.asarray(inputs["mask"])
    W1 = np.asarray(inputs["W1"], dtype=np.float32)
    b1 = np.asarray(inputs["b1"], dtype=np.float32)
    W2 = np.asarray(inputs["W2"], dtype=np.float32)
    b2 = np.asarray(inputs["b2"], dtype=np.float32)
    W3 = np.asarray(inputs["W3"], dtype=np.float32)

    Wt = W1[0:64] + W1[128:192]
    Ws = W1[64:128] - W1[128:192]
    Wm = W1[192:256]
    wsm = np.concatenate([Ws, Wm], axis=0).astype(bf16)
    # Fold U = t @ Wt' + b1 into the shipped data: solve wsm^T z = U
    # (least-norm, against the bf16-rounded device weights).
    wsm_f = wsm.astype(np.float32)
    Zmap = (wsm_f @ np.linalg.inv(wsm_f.T @ wsm_f)).astype(np.float32)  # [128, H1]
    U = tgt @ Wt + b1                                   # [B, H1] f32
    Z = U @ Zmap.T                                      # [B, 128]
    w2 = np.zeros((H1, 64), dtype=bf16)
    w2[:, 0:H2] = W2.astype(bf16)
    w3s2 = np.zeros((128, 64), dtype=bf16)
    w3s2[0:H2, 32] = W3[:, 0].astype(bf16)
    w3s2[64:64 + H2, 33] = W3[:, 0].astype(bf16)
    b2e = np.zeros((128, 1), dtype=np.float32)
    b2e[0:H2, 0] = b2
    b2e[64:64 + H2, 0] = b2

    maskb = mask.astype(bool)
    cnt = maskb.sum(1).astype(np.int64)
    assert cnt.max() <= CAPMAX, f"unmasked count {cnt.max()} exceeds {CAPMAX}"

    # sort rows by count within each core; superblock sb takes the sb-th
    # 128-row group, so its column cap is the group's max count (shared
    # across cores, rounded up to 8)
    cnt_c = cnt.reshape(NCORES, BPC)
    order_c = np.argsort(cnt_c, axis=1, kind="stable")   # [NCORES, BPC]
    caps = []
    for sb in range(NSUPER):
        grp = np.take_along_axis(
            cnt_c, order_c[:, sb * 128:(sb + 1) * 128], axis=1)
        caps.append(min(CAPMAX, int(-(-int(grp.max()) // 8) * 8)))
    caps = tuple(caps)

    # two device orderings: score-row order (count-sort only; amask, expm
    # and sums rows live here) and data order (count-sort composed with the
    # per-128-block pack permutation dev 4r+q <-> 32q+r; seqg/prodg rows)
    r_ = np.arange(128) // 4
    q_ = np.arange(128) % 4
    perm128 = 32 * q_ + r_
    score_rows = np.concatenate([
        core * BPC + order_c[core] for core in range(NCORES)])  # [B]
    assign = np.concatenate([
        core * BPC + order_c[core][sb * 128 + perm128]
        for core in range(NCORES) for sb in range(NSUPER)])   # [B] dev->orig

    # gather indices: unmasked positions first, padded with a masked slot
    order = np.argsort(~maskb, axis=1, kind="stable")   # unmasked first
    idx = order[:, :CAPMAX]
    pad_slot = order[:, -1]                             # guaranteed masked
    colpos = np.arange(CAPMAX)[None, :]
    idx = np.where(colpos < cnt[:, None], idx, pad_slot[:, None])

    # gathered feature-major seq+z1 and t*seq+z2 in device row order with
    # per-superblock widths, flattened to [64, TOT] per core
    g = np.take_along_axis(seq, idx[:, :, None], axis=1)     # [B, CAPMAX, D]
    sg = (g + Z[:, None, 0:64]).transpose(0, 2, 1).astype(bf16)       # [B, D, CAPMAX]
    pg = (g * tgt[:, None, :] + Z[:, None, 64:128]).transpose(0, 2, 1).astype(bf16)
    amask = np.where(colpos < cnt[:, None], 0.0, -1e9).astype(bf16)
    ident = np.eye(128, dtype=bf16)

    assign_c = assign.reshape(NCORES, BPC)
    TOT = CHUNK_B * sum(caps[c // 8] * 8 for c in range(NSUPER))  # per-core cols

    def flat_core(x, core):                      # x: [B, D, CAPMAX] -> [D, TOT]
        parts = []
        rows = assign_c[core]
        for c in range(NCHUNK):
            cp = caps[c // 8]
            blk = x[rows[c * CHUNK_B:(c + 1) * CHUNK_B], :, 0:cp]  # [16, D, cp]
            parts.append(blk.transpose(1, 0, 2).reshape(D, CHUNK_B * cp))
        return np.ascontiguousarray(np.concatenate(parts, axis=1))

    in_maps = []
    for core in range(NCORES):
        in_maps.append({
            "seqg": flat_core(sg, core),
            "prodg": flat_core(pg, core),
            "wsm": wsm,
            "w2": w2,
            "w3s2": w3s2,
            "b2e": b2e,
            "amaskb": amask[score_rows[core * BPC:(core + 1) * BPC]],
            "ident": ident,
        })
    return in_maps, idx, score_rows, caps


def kernel(**inputs) -> np.ndarray:
    from concourse.bass_utils import run_bass_kernel_spmd

    in_maps, idx, score_rows, caps = _host_prep(inputs)
    if caps not in _cache:
        _cache[caps] = _build_nc(caps)
    nc = _cache[caps]
    res = run_bass_kernel_spmd(nc, in_maps, list(range(NCORES)))
    expm = np.concatenate(
        [res.results[i]["out"] for i in range(NCORES)], axis=0)   # [B, CAPMAX] score order
    sums = np.concatenate(
        [res.results[i]["sums"] for i in range(NCORES)], axis=0)  # [B, 1]
    probs = expm / sums
    # columns beyond the row's superblock cap are uninitialized on device
    cap_row = np.tile(np.repeat(np.asarray(caps), 128), NCORES)   # [B]
    probs *= np.arange(CAPMAX)[None, :] < cap_row[:, None]
    out = np.zeros((B, S), dtype=np.float32)
    out[score_rows[:, None], idx[score_rows]] = probs
    return out


if __name__ == "__main__":
    rng = np.random.default_rng(0)
    fake = {
        "sequence_emb": rng.standard_normal((B, S, D), dtype=np.float32),
        "target_emb": rng.standard_normal((B, D), dtype=np.float32),
        "mask": rng.integers(0, 2, (B, S)).astype(np.int32),
        "W1": rng.standard_normal((4 * D, H1), dtype=np.float32) * 0.08,
        "b1": np.zeros(H1, np.float32),
        "W2": rng.standard_normal((H1, H2), dtype=np.float32) * 0.13,
        "b2": np.zeros(H2, np.float32),
        "W3": rng.standard_normal((H2, 1), dtype=np.float32) * 0.22,
        "b3": np.zeros(1, np.float32),
    }
    print(kernel(**fake).shape)
